# revision 10
# baseline (speedup 1.0000x reference)
"""Multi-head self-attention TRN2 Bass kernel (8 NeuronCores, SPMD).

Sharding: 8 cores = 4 batches x 2 query-halves. Each core gets its batch's
x pre-rotated so rows 0:1024 are its queries (keys/values are permutation
invariant), computes the full attention + output projection for those
queries, and writes its own disjoint slice of the output. No collectives.

Per-core program (Tile framework):
  phase 0: xT = x.T via PE transposes (fp32)
  phase 1: per head-pair (128 qkv channels): qT/kT/vT matmuls (bf16),
           v transposed back + a 64-wide all-ones block so the attention
           AV matmul also emits softmax denominators broadcast over 64
           partitions; scores^T -> exp(scale=1/8) on ScalarE -> AV
           accumulate -> normalize via exp(-ln(denom)) -> attnoutT
  phase 2: y = attnoutT.T @ w_proj, DMA out

mm_dt selects the matmul dtype: bfloat16 (default; ~649us/iter, rel err
~5e-3) or float32r (4-byte reduced-precision mode; ~803us, rel err 3e-4)
for a tighter-accuracy fallback.
"""

from contextlib import ExitStack
from dataclasses import dataclass

import numpy as np

import concourse.bass as bass
import concourse.mybir as mybir
import concourse.tile as tile
from concourse.bass_utils import run_bass_kernel_spmd
from concourse.masks import make_identity

FP32 = mybir.dt.float32
AF = mybir.ActivationFunctionType

N_CORES = 8
B, SEQ, DMODEL = 4, 2048, 1024


def _split_multi_waits(nc, max_waits=1):
    """This walrus build accepts at most one semaphore wait per instruction;
    split extra waits onto NoOp carriers inserted before the instruction."""
    for f in nc.m.functions:
        for blk in f.blocks:
            new = []
            for inst in blk.instructions:
                si = getattr(inst, "sync_info", None)
                waits = list(si.on_wait) if si is not None and si.on_wait else []
                if len(waits) > max_waits:
                    head, rest = waits[:-max_waits], waits[-max_waits:]
                    for i in range(0, len(head), max_waits):
                        nop = mybir.InstNoOp(
                            name=f"{inst.name}-sw{i}",
                            engine=inst.engine,
                            bass_nofuse=True,
                            sync_info=mybir.SyncInfo(
                                on_wait=head[i:i + max_waits], on_update=[]),
                        )
                        nc.register_instruction(nop, overwrite=True)
                        new.append(nop)
                    inst.sync_info = mybir.SyncInfo(
                        on_wait=rest, on_update=list(si.on_update or []))
                new.append(inst)
            blk.instructions = new


@dataclass
class _Cfg:
    T: int = 2048
    TQ: int = 1024
    D: int = 1024
    H: int = 16
    DH: int = 64
    NB: int = 512
    TQBS: int = 512
    mm_dt: mybir.dt = mybir.dt.bfloat16
    n_cores: int = N_CORES
    reps: int = 1    # hardware-loop repetitions of the whole kernel (timing)


def _build_nc(cfg):
    T, TQ, D, H, DH = cfg.T, cfg.TQ, cfg.D, cfg.H, cfg.DH
    NB, TQBS = cfg.NB, cfg.TQBS
    KT = D // 128
    NT = T // 128
    HP = H // 2
    TQB = TQ // TQBS
    SC = NT // 2
    QNB = TQ // NB
    TNB = T // NB
    DNB = D // NB
    DT = cfg.mm_dt

    nc = bass.Bass("TRN2", target_bir_lowering=False, debug=False,
                   num_devices=cfg.n_cores)
    x = nc.dram_tensor("x", [T, D], FP32, kind="ExternalInput")
    wdt = DT if mybir.dt.size(DT) == 4 else FP32
    w_qkv = nc.dram_tensor("w_qkv", [D, 3 * D], wdt, kind="ExternalInput")
    w_proj = nc.dram_tensor("w_proj", [D, D], wdt, kind="ExternalInput")
    y = nc.dram_tensor("y", [TQ, D], FP32, kind="ExternalOutput")

    scale = 1.0 / float(DH) ** 0.5

    with ExitStack() as outer:
        if cfg.reps > 1:
            outer.enter_context(nc.Fori(0, cfg.reps))
        _build_body(nc, cfg, x, w_qkv, w_proj, y)
    _split_multi_waits(nc)
    return nc


def _build_body(nc, cfg, x, w_qkv, w_proj, y):
    T, TQ, D, H, DH = cfg.T, cfg.TQ, cfg.D, cfg.H, cfg.DH
    NB, TQBS = cfg.NB, cfg.TQBS
    KT = D // 128
    NT = T // 128
    HP = H // 2
    TQB = TQ // TQBS
    SC = NT // 2
    QNB = TQ // NB
    TNB = T // NB
    DNB = D // NB
    DT = cfg.mm_dt
    scale = 1.0 / float(DH) ** 0.5

    with tile.TileContext(nc) as tc, ExitStack() as top:
        p_xT = top.enter_context(tc.tile_pool(name="xT", bufs=1))
        p_attn = top.enter_context(tc.tile_pool(name="attnoutT", bufs=1))
        p_const = top.enter_context(tc.tile_pool(name="const", bufs=1))
        p_mm = top.enter_context(tc.tile_pool(name="ps_mm", bufs=2, space="PSUM"))
        p_s = top.enter_context(tc.tile_pool(name="ps_scores", bufs=2, space="PSUM"))
        p_o = top.enter_context(tc.tile_pool(name="ps_out", bufs=2, space="PSUM"))

        ident = p_const.tile([128, 128], FP32)
        make_identity(nc, ident[:])
        identb = p_const.tile([128, 128], DT)
        if mybir.dt.size(DT) == 2:
            make_identity(nc, identb[:])
        ones = p_const.tile([128, NT * DH], FP32)
        nc.vector.memset(ones[:], 1.0)

        xT = p_xT.tile([128, KT, T], DT)
        attnoutT = p_attn.tile([128, KT, TQ], DT)

        # phase 0: x -> xT
        two_b = mybir.dt.size(DT) == 2
        with tc.tile_pool(name="x_nat", bufs=3) as p_xn:
            for t in range(NT):
                x_nat = p_xn.tile([128, D], DT if two_b else FP32, tag="xn")
                (nc.gpsimd if two_b else nc.sync).dma_start(
                    out=x_nat[:], in_=x[t * 128:(t + 1) * 128, :])
                for k in range(KT):
                    ps_tr = p_mm.tile([128, 128], DT if two_b else FP32, tag="ps")
                    nc.tensor.transpose(ps_tr[:], x_nat[:, k * 128:(k + 1) * 128],
                                        identb[:] if two_b else ident[:])
                    nc.vector.tensor_copy(
                        xT[:, k, t * 128:(t + 1) * 128], ps_tr[:])

        # phase 1
        with ExitStack() as hps:
            p_w = hps.enter_context(tc.tile_pool(name="w", bufs=2))
            p_qT = hps.enter_context(tc.tile_pool(name="qT", bufs=2))
            p_kT = hps.enter_context(tc.tile_pool(name="kT", bufs=2))
            p_vT = hps.enter_context(tc.tile_pool(name="vT", bufs=2))
            p_aug = hps.enter_context(tc.tile_pool(name="aug", bufs=2))
            p_exp = hps.enter_context(tc.tile_pool(name="expsT", bufs=4))
            p_sm = hps.enter_context(tc.tile_pool(name="small", bufs=3))

            for hp in range(HP):
                w_hp = p_w.tile([128, 3, KT, 128], DT, tag="w")
                w_eng = nc.sync if w_qkv.dtype == DT else nc.gpsimd
                for j, base in enumerate((0, D, 2 * D)):
                    c0 = base + hp * 128
                    for k in range(KT):
                        w_eng.dma_start(
                            out=w_hp[:, j, k, :],
                            in_=w_qkv[k * 128:(k + 1) * 128, c0:c0 + 128])

                qT = p_qT.tile([128, TQ], DT, tag="qT")
                kT = p_kT.tile([128, T], DT, tag="kT")
                vT = p_vT.tile([128, T], DT, tag="vT")
                for dst, wj, nblk in ((qT, 0, QNB), (kT, 1, TNB), (vT, 2, TNB)):
                    for b in range(nblk):
                        ps = p_mm.tile([128, NB], FP32, tag="ps")
                        for k in range(KT):
                            nc.tensor.matmul(
                                ps[:], w_hp[:, wj, k, :],
                                xT[:, k, b * NB:(b + 1) * NB],
                                start=(k == 0), stop=(k == KT - 1))
                        nc.vector.tensor_copy(dst[:, b * NB:(b + 1) * NB], ps[:])

                augA = p_aug.tile([128, NT, 2 * DH], DT, tag="augA")
                augB = p_aug.tile([128, NT, 2 * DH], DT, tag="augB")
                ones3 = ones[:].rearrange("p (t d) -> p t d", t=NT)
                nc.vector.tensor_copy(augA[:, :, DH:2 * DH], ones3)
                nc.vector.tensor_copy(augB[:, :, DH:2 * DH], ones3)
                for t in range(NT):
                    ps_tr = p_mm.tile([128, 128], DT if two_b else FP32, tag="ps")
                    nc.tensor.transpose(ps_tr[:], vT[:, t * 128:(t + 1) * 128],
                                        identb[:] if two_b else ident[:])
                    nc.vector.tensor_copy(augA[:, t, 0:DH], ps_tr[:, 0:DH])
                    nc.vector.tensor_copy(augB[:, t, 0:DH], ps_tr[:, DH:128])

                for tb in range(TQB):
                    q0 = tb * TQBS
                    oA = p_o.tile([128, TQBS], FP32, tag="o")
                    oB = p_o.tile([128, TQBS], FP32, tag="o")
                    for tk in range(NT):
                        # one psum tile holds both heads' score blocks, so the
                        # two K=64 matmuls are adjacent with no slot wait
                        # between them -> PE runs them packed in disjoint row
                        # groups (base partitions 0 / 64).
                        ps_p = p_s.tile([128, 2, TQBS], FP32, tag="s")
                        exps = p_exp.tile([128, 2, TQBS], DT, tag="e")
                        nc.tensor.matmul(
                            ps_p[:, 0, :],
                            kT[0:DH, tk * 128:(tk + 1) * 128],
                            qT[0:DH, q0:q0 + TQBS],
                            start=True, stop=True)
                        nc.tensor.matmul(
                            ps_p[:, 1, :],
                            kT[DH:128, tk * 128:(tk + 1) * 128],
                            qT[DH:128, q0:q0 + TQBS],
                            start=True, stop=True)
                        nc.scalar.activation(exps[:], ps_p[:], AF.Exp, scale=scale)
                        nc.tensor.matmul(
                            oA[:, :], augA[:, tk, :], exps[:, 0, :],
                            start=(tk == 0), stop=(tk == NT - 1))
                        nc.tensor.matmul(
                            oB[:, :], augB[:, tk, :], exps[:, 1, :],
                            start=(tk == 0), stop=(tk == NT - 1))

                    # normalize: rows 0:DH values, DH:128 denominator (repl.)
                    for o, shift in ((oA, False), (oB, True)):
                        ln_t = p_sm.tile([128, TQBS], FP32, tag="lnr")
                        r_t = p_sm.tile([128, TQBS], FP32, tag="lnr")
                        rs_t = p_sm.tile([DH, TQBS], FP32, tag="rs")
                        nc.scalar.activation(ln_t[DH:128, :], o[DH:128, :], AF.Ln)
                        nc.scalar.activation(r_t[DH:128, :], ln_t[DH:128, :],
                                             AF.Exp, scale=-1.0)
                        nc.sync.dma_start(out=rs_t[:], in_=r_t[DH:128, :])
                        if not shift:
                            nc.vector.tensor_mul(
                                attnoutT[0:DH, hp, q0:q0 + TQBS],
                                o[0:DH, :], rs_t[:])
                        else:
                            stB = p_sm.tile([DH, TQBS], DT, tag="st")
                            nc.vector.tensor_mul(stB[:], o[0:DH, :], rs_t[:])
                            nc.sync.dma_start(
                                out=attnoutT[DH:128, hp, q0:q0 + TQBS], in_=stB[:])

        # phase 2: y = attnoutT.T @ w_proj
        with tc.tile_pool(name="wp", bufs=2) as p_wp, \
             tc.tile_pool(name="ystage", bufs=3) as p_y:
            for nb in range(DNB):
                wp = p_wp.tile([128, KT, NB], DT, tag="wp")
                wp_eng = nc.sync if w_proj.dtype == DT else nc.gpsimd
                for k in range(KT):
                    wp_eng.dma_start(
                        out=wp[:, k, :],
                        in_=w_proj[k * 128:(k + 1) * 128, nb * NB:(nb + 1) * NB])
                for tt in range(TQ // 128):
                    ps = p_mm.tile([128, NB], FP32, tag="ps")
                    for k in range(KT):
                        nc.tensor.matmul(
                            ps[:], attnoutT[:, k, tt * 128:(tt + 1) * 128],
                            wp[:, k, :],
                            start=(k == 0), stop=(k == KT - 1))
                    ys = p_y.tile([128, NB], FP32, tag="ys")
                    nc.vector.tensor_copy(ys[:], ps[:])
                    nc.sync.dma_start(
                        out=y[tt * 128:(tt + 1) * 128, nb * NB:(nb + 1) * NB],
                        in_=ys[:])


_NC_CACHE = {}


def _get_nc():
    if "nc" not in _NC_CACHE:
        _NC_CACHE["nc"] = _build_nc(_Cfg())
    return _NC_CACHE["nc"]


def make_in_maps(x, w_qkv, w_proj, tq=1024):
    x = np.ascontiguousarray(np.asarray(x, dtype=np.float32))
    w_qkv = np.ascontiguousarray(np.asarray(w_qkv, dtype=np.float32))
    w_proj = np.ascontiguousarray(np.asarray(w_proj, dtype=np.float32))
    in_maps = []
    for core in range(N_CORES):
        b, half = divmod(core, 2)
        x_rot = np.concatenate([x[b, half * tq:], x[b, :half * tq]], axis=0)
        in_maps.append({"x": np.ascontiguousarray(x_rot),
                        "w_qkv": w_qkv, "w_proj": w_proj})
    return in_maps


def assemble(results, tq=1024):
    out = np.empty((B, SEQ, DMODEL), np.float32)
    for core in range(N_CORES):
        b, half = divmod(core, 2)
        out[b, half * tq:(half + 1) * tq] = results[core]["y"]
    return out


def kernel(x, w_qkv, w_proj):
    nc = _get_nc()
    in_maps = make_in_maps(x, w_qkv, w_proj)
    res = run_bass_kernel_spmd(nc, in_maps, list(range(N_CORES)))
    return assemble(res.results)


# revision 12
# speedup vs baseline: 1.0896x; 1.0896x over previous
"""Multi-head self-attention TRN2 Bass kernel (8 NeuronCores, SPMD).

Sharding: 8 cores = 4 batches x 2 query-halves. Each core gets its batch's
x pre-rotated so rows 0:1024 are its queries (keys/values are permutation
invariant), computes the full attention + output projection for those
queries, and writes its own disjoint slice of the output. No collectives.

Per-core program (Tile framework):
  phase 0: xT = x.T via PE transposes (fp32)
  phase 1: per head-pair (128 qkv channels): qT/kT/vT matmuls (bf16),
           v transposed back + a 64-wide all-ones block so the attention
           AV matmul also emits softmax denominators broadcast over 64
           partitions; scores^T -> exp(scale=1/8) on ScalarE -> AV
           accumulate -> normalize via exp(-ln(denom)) -> attnoutT
  phase 2: y = attnoutT.T @ w_proj, DMA out

mm_dt selects the matmul dtype: bfloat16 (default; ~649us/iter, rel err
~5e-3) or float32r (4-byte reduced-precision mode; ~803us, rel err 3e-4)
for a tighter-accuracy fallback.
"""

from contextlib import ExitStack
from dataclasses import dataclass

import numpy as np

import concourse.bass as bass
import concourse.mybir as mybir
import concourse.tile as tile
from concourse.bass_utils import run_bass_kernel_spmd
from concourse.masks import make_identity

FP32 = mybir.dt.float32
AF = mybir.ActivationFunctionType

N_CORES = 8
B, SEQ, DMODEL = 4, 2048, 1024


def _split_multi_waits(nc, max_waits=1):
    """This walrus build accepts at most one semaphore wait per instruction;
    split extra waits onto NoOp carriers inserted before the instruction."""
    for f in nc.m.functions:
        for blk in f.blocks:
            new = []
            for inst in blk.instructions:
                si = getattr(inst, "sync_info", None)
                waits = list(si.on_wait) if si is not None and si.on_wait else []
                if len(waits) > max_waits:
                    head, rest = waits[:-max_waits], waits[-max_waits:]
                    for i in range(0, len(head), max_waits):
                        nop = mybir.InstNoOp(
                            name=f"{inst.name}-sw{i}",
                            engine=inst.engine,
                            bass_nofuse=True,
                            sync_info=mybir.SyncInfo(
                                on_wait=head[i:i + max_waits], on_update=[]),
                        )
                        nc.register_instruction(nop, overwrite=True)
                        new.append(nop)
                    inst.sync_info = mybir.SyncInfo(
                        on_wait=rest, on_update=list(si.on_update or []))
                new.append(inst)
            blk.instructions = new


@dataclass
class _Cfg:
    T: int = 2048
    TQ: int = 1024
    D: int = 1024
    H: int = 16
    DH: int = 64
    NB: int = 512
    TQBS: int = 512
    mm_dt: mybir.dt = mybir.dt.bfloat16
    n_cores: int = N_CORES
    reps: int = 1    # hardware-loop repetitions of the whole kernel (timing)


def _build_nc(cfg):
    T, TQ, D, H, DH = cfg.T, cfg.TQ, cfg.D, cfg.H, cfg.DH
    NB, TQBS = cfg.NB, cfg.TQBS
    KT = D // 128
    NT = T // 128
    HP = H // 2
    TQB = TQ // TQBS
    SC = NT // 2
    QNB = TQ // NB
    TNB = T // NB
    DNB = D // NB
    DT = cfg.mm_dt

    nc = bass.Bass("TRN2", target_bir_lowering=False, debug=False,
                   num_devices=cfg.n_cores)
    x = nc.dram_tensor("x", [T, D], FP32, kind="ExternalInput")
    wdt = DT if mybir.dt.size(DT) == 4 else FP32
    w_qkv = nc.dram_tensor("w_qkv", [D, 3 * D], wdt, kind="ExternalInput")
    w_proj = nc.dram_tensor("w_proj", [D, D], wdt, kind="ExternalInput")
    y = nc.dram_tensor("y", [TQ, D], FP32, kind="ExternalOutput")

    scale = 1.0 / float(DH) ** 0.5

    with ExitStack() as outer:
        if cfg.reps > 1:
            outer.enter_context(nc.Fori(0, cfg.reps))
        _build_body(nc, cfg, x, w_qkv, w_proj, y)
    _split_multi_waits(nc)
    return nc


def _build_body(nc, cfg, x, w_qkv, w_proj, y):
    T, TQ, D, H, DH = cfg.T, cfg.TQ, cfg.D, cfg.H, cfg.DH
    NB, TQBS = cfg.NB, cfg.TQBS
    KT = D // 128
    NT = T // 128
    HP = H // 2
    TQB = TQ // TQBS
    SC = NT // 2
    QNB = TQ // NB
    TNB = T // NB
    DNB = D // NB
    DT = cfg.mm_dt
    scale = 1.0 / float(DH) ** 0.5

    with tile.TileContext(nc) as tc, ExitStack() as top:
        p_xT = top.enter_context(tc.tile_pool(name="xT", bufs=1))
        p_attn = top.enter_context(tc.tile_pool(name="attnoutT", bufs=1))
        p_const = top.enter_context(tc.tile_pool(name="const", bufs=1))
        p_mm = top.enter_context(tc.tile_pool(name="ps_mm", bufs=2, space="PSUM"))
        p_s = top.enter_context(tc.tile_pool(name="ps_scores", bufs=2, space="PSUM"))
        p_o = top.enter_context(tc.tile_pool(name="ps_out", bufs=2, space="PSUM"))

        ident = p_const.tile([128, 128], FP32)
        make_identity(nc, ident[:])
        identb = p_const.tile([128, 128], DT)
        if mybir.dt.size(DT) == 2:
            make_identity(nc, identb[:])
        ones = p_const.tile([128, NT * DH], FP32)
        nc.vector.memset(ones[:], 1.0)

        xT = p_xT.tile([128, KT, T], DT)
        attnoutT = p_attn.tile([128, KT, TQ], DT)

        # phase 0: x -> xT. k-outer order so each xT d-slice completes as
        # early as possible, letting the first qkv accumulation chains start
        # while later slices are still being transposed.
        two_b = mybir.dt.size(DT) == 2
        with tc.tile_pool(name="x_nat", bufs=1 if two_b else 3) as p_xn:
            if two_b:
                x_nats = []
                for t in range(NT):
                    xn = p_xn.tile([128, D], DT, tag=f"xn{t}")
                    nc.gpsimd.dma_start(
                        out=xn[:], in_=x[t * 128:(t + 1) * 128, :])
                    x_nats.append(xn)
                for k in range(KT):
                    for t in range(NT):
                        ps_tr = p_mm.tile([128, 128], DT, tag="ps")
                        nc.tensor.transpose(
                            ps_tr[:], x_nats[t][:, k * 128:(k + 1) * 128],
                            identb[:])
                        nc.vector.tensor_copy(
                            xT[:, k, t * 128:(t + 1) * 128], ps_tr[:])
            else:
                for t in range(NT):
                    x_nat = p_xn.tile([128, D], FP32, tag="xn")
                    nc.sync.dma_start(
                        out=x_nat[:], in_=x[t * 128:(t + 1) * 128, :])
                    for k in range(KT):
                        ps_tr = p_mm.tile([128, 128], FP32, tag="ps")
                        nc.tensor.transpose(
                            ps_tr[:], x_nat[:, k * 128:(k + 1) * 128], ident[:])
                        nc.vector.tensor_copy(
                            xT[:, k, t * 128:(t + 1) * 128], ps_tr[:])

        # phase 1
        with ExitStack() as hps:
            p_w = hps.enter_context(tc.tile_pool(name="w", bufs=2))
            p_qT = hps.enter_context(tc.tile_pool(name="qT", bufs=2))
            p_kT = hps.enter_context(tc.tile_pool(name="kT", bufs=2))
            p_vT = hps.enter_context(tc.tile_pool(name="vT", bufs=2))
            p_aug = hps.enter_context(tc.tile_pool(name="aug", bufs=2))
            p_exp = hps.enter_context(tc.tile_pool(name="expsT", bufs=4))
            p_sm = hps.enter_context(tc.tile_pool(name="small", bufs=3))

            for hp in range(HP):
                w_hp = p_w.tile([128, 3, KT, 128], DT, tag="w")
                w_eng = nc.sync if w_qkv.dtype == DT else nc.gpsimd
                for j, base in enumerate((0, D, 2 * D)):
                    c0 = base + hp * 128
                    for k in range(KT):
                        w_eng.dma_start(
                            out=w_hp[:, j, k, :],
                            in_=w_qkv[k * 128:(k + 1) * 128, c0:c0 + 128])

                qT = p_qT.tile([128, TQ], DT, tag="qT")
                kT = p_kT.tile([128, T], DT, tag="kT")
                vT = p_vT.tile([128, T], DT, tag="vT")
                for dst, wj, nblk in ((qT, 0, QNB), (kT, 1, TNB), (vT, 2, TNB)):
                    for b in range(nblk):
                        ps = p_mm.tile([128, NB], FP32, tag="ps")
                        for k in range(KT):
                            nc.tensor.matmul(
                                ps[:], w_hp[:, wj, k, :],
                                xT[:, k, b * NB:(b + 1) * NB],
                                start=(k == 0), stop=(k == KT - 1))
                        nc.vector.tensor_copy(dst[:, b * NB:(b + 1) * NB], ps[:])

                augA = p_aug.tile([128, NT, 2 * DH], DT, tag="augA")
                augB = p_aug.tile([128, NT, 2 * DH], DT, tag="augB")
                ones3 = ones[:].rearrange("p (t d) -> p t d", t=NT)
                nc.vector.tensor_copy(augA[:, :, DH:2 * DH], ones3)
                nc.vector.tensor_copy(augB[:, :, DH:2 * DH], ones3)
                for t in range(NT):
                    ps_tr = p_mm.tile([128, 128], DT if two_b else FP32, tag="ps")
                    nc.tensor.transpose(ps_tr[:], vT[:, t * 128:(t + 1) * 128],
                                        identb[:] if two_b else ident[:])
                    nc.vector.tensor_copy(augA[:, t, 0:DH], ps_tr[:, 0:DH])
                    nc.vector.tensor_copy(augB[:, t, 0:DH], ps_tr[:, DH:128])

                for tb in range(TQB):
                    q0 = tb * TQBS
                    oA = p_o.tile([128, TQBS], FP32, tag="o")
                    oB = p_o.tile([128, TQBS], FP32, tag="o")
                    for tk in range(NT):
                        # one psum tile holds both heads' score blocks, so the
                        # two K=64 matmuls are adjacent with no slot wait
                        # between them -> PE runs them packed in disjoint row
                        # groups (base partitions 0 / 64).
                        ps_p = p_s.tile([128, 2, TQBS], FP32, tag="s")
                        exps = p_exp.tile([128, 2, TQBS], DT, tag="e")
                        nc.tensor.matmul(
                            ps_p[:, 0, :],
                            kT[0:DH, tk * 128:(tk + 1) * 128],
                            qT[0:DH, q0:q0 + TQBS],
                            start=True, stop=True)
                        nc.tensor.matmul(
                            ps_p[:, 1, :],
                            kT[DH:128, tk * 128:(tk + 1) * 128],
                            qT[DH:128, q0:q0 + TQBS],
                            start=True, stop=True)
                        nc.scalar.activation(exps[:], ps_p[:], AF.Exp, scale=scale)
                        nc.tensor.matmul(
                            oA[:, :], augA[:, tk, :], exps[:, 0, :],
                            start=(tk == 0), stop=(tk == NT - 1))
                        nc.tensor.matmul(
                            oB[:, :], augB[:, tk, :], exps[:, 1, :],
                            start=(tk == 0), stop=(tk == NT - 1))

                    # normalize: rows 0:DH values, DH:128 denominator (repl.)
                    for o, shift in ((oA, False), (oB, True)):
                        ln_t = p_sm.tile([128, TQBS], FP32, tag="lnr")
                        r_t = p_sm.tile([128, TQBS], FP32, tag="lnr")
                        rs_t = p_sm.tile([DH, TQBS], FP32, tag="rs")
                        nc.scalar.activation(ln_t[DH:128, :], o[DH:128, :], AF.Ln)
                        nc.scalar.activation(r_t[DH:128, :], ln_t[DH:128, :],
                                             AF.Exp, scale=-1.0)
                        nc.sync.dma_start(out=rs_t[:], in_=r_t[DH:128, :])
                        if not shift:
                            nc.vector.tensor_mul(
                                attnoutT[0:DH, hp, q0:q0 + TQBS],
                                o[0:DH, :], rs_t[:])
                        else:
                            stB = p_sm.tile([DH, TQBS], DT, tag="st")
                            nc.vector.tensor_mul(stB[:], o[0:DH, :], rs_t[:])
                            nc.sync.dma_start(
                                out=attnoutT[DH:128, hp, q0:q0 + TQBS], in_=stB[:])

        # phase 2: y = attnoutT.T @ w_proj
        with tc.tile_pool(name="wp", bufs=2) as p_wp, \
             tc.tile_pool(name="ystage", bufs=3) as p_y:
            for nb in range(DNB):
                wp = p_wp.tile([128, KT, NB], DT, tag="wp")
                wp_eng = nc.sync if w_proj.dtype == DT else nc.gpsimd
                for k in range(KT):
                    wp_eng.dma_start(
                        out=wp[:, k, :],
                        in_=w_proj[k * 128:(k + 1) * 128, nb * NB:(nb + 1) * NB])
                for tt in range(TQ // 128):
                    ps = p_mm.tile([128, NB], FP32, tag="ps")
                    for k in range(KT):
                        nc.tensor.matmul(
                            ps[:], attnoutT[:, k, tt * 128:(tt + 1) * 128],
                            wp[:, k, :],
                            start=(k == 0), stop=(k == KT - 1))
                    ys = p_y.tile([128, NB], FP32, tag="ys")
                    nc.vector.tensor_copy(ys[:], ps[:])
                    nc.sync.dma_start(
                        out=y[tt * 128:(tt + 1) * 128, nb * NB:(nb + 1) * NB],
                        in_=ys[:])


_NC_CACHE = {}


def _get_nc():
    if "nc" not in _NC_CACHE:
        _NC_CACHE["nc"] = _build_nc(_Cfg())
    return _NC_CACHE["nc"]


def make_in_maps(x, w_qkv, w_proj, tq=1024):
    x = np.ascontiguousarray(np.asarray(x, dtype=np.float32))
    w_qkv = np.ascontiguousarray(np.asarray(w_qkv, dtype=np.float32))
    w_proj = np.ascontiguousarray(np.asarray(w_proj, dtype=np.float32))
    in_maps = []
    for core in range(N_CORES):
        b, half = divmod(core, 2)
        x_rot = np.concatenate([x[b, half * tq:], x[b, :half * tq]], axis=0)
        in_maps.append({"x": np.ascontiguousarray(x_rot),
                        "w_qkv": w_qkv, "w_proj": w_proj})
    return in_maps


def assemble(results, tq=1024):
    out = np.empty((B, SEQ, DMODEL), np.float32)
    for core in range(N_CORES):
        b, half = divmod(core, 2)
        out[b, half * tq:(half + 1) * tq] = results[core]["y"]
    return out


def kernel(x, w_qkv, w_proj):
    nc = _get_nc()
    in_maps = make_in_maps(x, w_qkv, w_proj)
    res = run_bass_kernel_spmd(nc, in_maps, list(range(N_CORES)))
    return assemble(res.results)
